# revision 1
# baseline (speedup 1.0000x reference)
"""Multi-head causal attention on 8 Trainium2 NeuronCores.

Sharding: data-parallel over batch (4) x tensor-parallel over heads (2 groups
of 8 heads). Each core computes a partial output [T, C] for one batch element
using its 8 heads; the host sums the two partials per batch element (the
"all-reduce after out_proj" done during unshard).

Per-core algorithm (all layouts chosen so no on-device transposes are needed):
  inputs: xT [C, T] (x[b] transposed on host), Wq/Wk/Wv [C, 512], Wo [512, C],
          causal multiplicative masks [4, 128, 512] (bf16).
  QT = Wq^T @ x^T * 1/sqrt(Dh)  [512, T]   (lhsT = Wq chunk, rhs = xT chunk)
  KT = Wk^T @ x^T               [512, T]
  V  = x @ Wv                   [T, 512]   (lhsT = xT chunk, rhs = Wv)
       stored ones-augmented as V_aug [T, 8 heads, 65] bf16 (col 64 = 1.0)
  per head h, query-chunk j (512 wide), key-block kb (128 wide, causal only):
     sT  = K_h[kb]^T @ Q_h[:, j]           [128, 512] PSUM  (fp32r matmul)
     p   = exp(sT)                         bf16 (skip-max softmax: |s| < ~9)
     p  *= mask                            (diagonal blocks only)
     av += V_aug[kb, h]^T @ p              [65, 512] PSUM; row 64 = denom
  attn_outT[h, :, j] = av[:64] * (1/denom broadcast via ones-matmul)
  out[tb] = attn_outT[:, tb]^T @ Wo        [128, 1024] -> partial output
"""

import numpy as np
import ml_dtypes

_BF = ml_dtypes.bfloat16

import concourse.bass as bass
import concourse.bacc as bacc
import concourse.mybir as mybir
import concourse.tile as tile
from concourse import bass_utils

F32 = mybir.dt.float32
F32R = mybir.dt.float32r
BF16 = mybir.dt.bfloat16

B, T, C = 4, 2048, 1024
H, Dh = 16, 64
G = 2                 # head groups (tensor parallel)
HPG = H // G          # heads per group
GC = HPG * Dh         # group channels = 512
N_CORES = 8
TC = 512              # token chunk (phase 2 and query chunks)
KB = 128              # key block
N_TC = T // TC        # 4
N_KB = T // KB        # 16
N_CC = C // 128       # contraction chunks over C = 8
N_GCB = GC // 128     # chan blocks in a group = 4


def build_program():
    nc = bacc.Bacc("TRN2", target_bir_lowering=False, debug=False)

    xT = nc.dram_tensor("xT", [C, T], BF16, kind="ExternalInput").ap()
    wq = nc.dram_tensor("wq", [C, GC], BF16, kind="ExternalInput").ap()
    wk = nc.dram_tensor("wk", [C, GC], BF16, kind="ExternalInput").ap()
    wv = nc.dram_tensor("wv", [C, GC], BF16, kind="ExternalInput").ap()
    wo = nc.dram_tensor("wo", [GC, C], BF16, kind="ExternalInput").ap()
    masks = nc.dram_tensor("masks", [4, KB, TC], BF16, kind="ExternalInput").ap()
    ones_in = nc.dram_tensor("ones", [1, Dh], F32R, kind="ExternalInput").ap()
    sel_in = nc.dram_tensor("sel", [32, 32 * Dh], F32R, kind="ExternalInput").ap()
    out = nc.dram_tensor("out", [T, C], F32, kind="ExternalOutput").ap()

    with tile.TileContext(nc) as tc:
        with tc.tile_pool(name="persist", bufs=1) as pp:
            qt = pp.tile([128, N_GCB, T], BF16)        # QT (chan%128, chan//128, tok)
            kt = pp.tile([128, N_GCB, T], BF16)
            vaug = pp.tile([128, N_KB, HPG, Dh + 1], BF16)
            aot = pp.tile([128, N_GCB, T], BF16)       # attn_outT
            msk = pp.tile([128, 4, TC], BF16)
            ones = pp.tile([1, Dh], F32R)
            sel = pp.tile([32, 32 * Dh], F32R)

            nc.sync.dma_start(msk[:], masks.rearrange("m p n -> p m n"))
            nc.sync.dma_start(ones[:], ones_in)
            nc.sync.dma_start(sel[:], sel_in)
            nc.vector.memset(vaug[:, :, :, Dh:], 1.0)

            # ---------------- phase 2: qkv projections -----------------
            with (
                tc.tile_pool(name="wq_pool", bufs=1) as wqp,
                tc.tile_pool(name="x_pool", bufs=2) as xp,
                tc.tile_pool(name="proj_psum", bufs=4, space="PSUM") as pjp,
            ):
                wqs = wqp.tile([128, N_CC, GC], BF16, tag="wq")
                wks = wqp.tile([128, N_CC, GC], BF16, tag="wk")
                wvs = wqp.tile([128, N_CC, GC], BF16, tag="wv")
                nc.sync.dma_start(wqs[:], wq.rearrange("(kc p) n -> p kc n", p=128))
                nc.sync.dma_start(wks[:], wk.rearrange("(kc p) n -> p kc n", p=128))
                nc.sync.dma_start(wvs[:], wv.rearrange("(kc p) n -> p kc n", p=128))

                for t in range(N_TC):
                    xt = xp.tile([128, N_CC, TC], BF16, tag="xt")
                    nc.sync.dma_start(
                        xt[:],
                        xT[:, t * TC:(t + 1) * TC].rearrange(
                            "(kc p) n -> p kc n", p=128
                        ),
                    )
                    for oc in range(N_GCB):      # QT and KT column blocks
                        for w_s, dst, scale in ((wqs, qt, 0.125), (wks, kt, None)):
                            ps = pjp.tile([128, TC], F32, tag="pj")
                            for kc in range(N_CC):
                                nc.tensor.matmul(
                                    ps[:],
                                    w_s[:, kc, oc * 128:(oc + 1) * 128],
                                    xt[:, kc, :],
                                    start=(kc == 0),
                                    stop=(kc == N_CC - 1),
                                )
                            dslc = dst[:, oc, t * TC:(t + 1) * TC]
                            if scale is None:
                                nc.vector.tensor_copy(dslc, ps[:])
                            else:
                                nc.vector.tensor_scalar_mul(dslc, ps[:], scale)
                    for tb in range(TC // 128):  # V token blocks
                        ps = pjp.tile([128, GC], F32, tag="pj")
                        for kc in range(N_CC):
                            nc.tensor.matmul(
                                ps[:],
                                xt[:, kc, tb * 128:(tb + 1) * 128],
                                wvs[:, kc, :],
                                start=(kc == 0),
                                stop=(kc == N_CC - 1),
                            )
                        nc.vector.tensor_copy(
                            vaug[:, t * 4 + tb, :, :Dh],
                            ps.rearrange("p (h d) -> p h d", h=HPG),
                        )

            # ---------------- phase 3: attention -----------------------
            # Softmax denominator rows staged for batched normalization.
            # Engine APs may only start at partitions {0,32,64}, so slot s
            # lives at (partition 32*(s//11), column s%11); a DMA later
            # compacts the slots into a [32, TC] tile for one reciprocal.
            lctx = tc.tile_pool(name="ph3_long", bufs=1)
            lp = lctx.__enter__()
            dens = lp.tile([65, 11, TC], F32)
            with (
                tc.tile_pool(name="probs", bufs=4) as prp,
                tc.tile_pool(name="sc_psum", bufs=2, space="PSUM") as scp,
                tc.tile_pool(name="av_psum", bufs=4, space="PSUM") as avp,
            ):
                for j in range(N_TC):            # query chunk
                    qslc = slice(j * TC, (j + 1) * TC)
                    for p in range(HPG // 2):    # head pairs: rows 0:64 / 64:128
                        avs = [
                            avp.tile([Dh + 1, TC], F32, tag="av", name=f"av{i}")
                            for i in range(2)
                        ]
                        nkb = 4 * j + 4
                        for kb in range(nkb):
                            # both heads' score tiles side by side in one
                            # 2-bank PSUM tile -> single exp op per kb
                            sc = scp.tile([128, 2 * TC], F32, tag="sc")
                            for half in range(2):
                                p0 = half * Dh
                                nc.tensor.matmul(
                                    sc[:, half * TC:(half + 1) * TC],
                                    kt[p0:p0 + Dh, p, kb * KB:(kb + 1) * KB],
                                    qt[p0:p0 + Dh, p, qslc],
                                    start=True,
                                    stop=True,
                                )
                            pr = prp.tile([128, 2 * TC], BF16, tag="pr")
                            nc.scalar.activation(
                                pr[:], sc[:], mybir.ActivationFunctionType.Exp
                            )
                            m = kb - 4 * j
                            if m >= 0:
                                for half in range(2):
                                    nc.vector.tensor_mul(
                                        pr[:, half * TC:(half + 1) * TC],
                                        pr[:, half * TC:(half + 1) * TC],
                                        msk[:, m, :],
                                    )
                            for half in range(2):
                                nc.tensor.matmul(
                                    avs[half][:],
                                    vaug[:, kb, 2 * p + half, :],
                                    pr[:, half * TC:(half + 1) * TC],
                                    start=(kb == 0),
                                    stop=(kb == nkb - 1),
                                )
                        for half in range(2):
                            p0 = half * Dh
                            idx = (j * 4 + p) * 2 + half
                            nc.vector.tensor_copy(
                                aot[p0:p0 + Dh, p, qslc], avs[half][:Dh, :]
                            )
                            db, dc = 32 * (idx // 11), idx % 11
                            nc.vector.tensor_copy(
                                dens[db:db + 1, dc, :], avs[half][Dh:Dh + 1, :]
                            )

            # tail: one reciprocal over all 32 denominator rows, then per
            # query-chunk j: selection-matmul broadcasts + in-place multiply
            # on aot immediately followed by that j's output projection --
            # keeps PE dense (warm) through the end of the kernel.
            with (
                tc.tile_pool(name="outs", bufs=3) as otp,
                tc.tile_pool(name="wo_pool", bufs=1) as wop,
                tc.tile_pool(name="bc_psum", bufs=2, space="PSUM") as bcp,
                tc.tile_pool(name="out_psum", bufs=2, space="PSUM") as opp,
            ):
                wos = wop.tile([128, N_GCB, C], BF16)
                nc.sync.dma_start(wos[:], wo.rearrange("(cb p) n -> p cb n", p=128))
                dcomp = lp.tile([32, TC], F32)
                for b in range(3):
                    lo, n = 11 * b, min(11, 32 - 11 * b)
                    nc.sync.dma_start(
                        dcomp[lo:lo + n, :], dens[32 * b:32 * b + 1, :n, :]
                    )
                rec = lp.tile([32, TC], F32R)
                with nc.allow_low_precision(
                    reason="fp32r reciprocal feeds bcast matmul"
                ):
                    nc.vector.reciprocal(rec[:], dcomp[:])
                for j in range(N_TC):
                    qslc = slice(j * TC, (j + 1) * TC)
                    for p in range(HPG // 2):
                        for half in range(2):
                            p0 = half * Dh
                            idx = (j * 4 + p) * 2 + half
                            bc = bcp.tile([Dh, TC], F32, tag="bc")
                            nc.tensor.matmul(
                                bc[:],
                                sel[:, idx * Dh:(idx + 1) * Dh],
                                rec[:],
                                start=True, stop=True,
                            )
                            nc.vector.tensor_mul(
                                aot[p0:p0 + Dh, p, qslc],
                                aot[p0:p0 + Dh, p, qslc],
                                bc[:],
                            )
                    for tb in range(4 * j, 4 * j + 4):
                        ot = otp.tile([128, C], F32, tag="ot")
                        for oc in range(C // TC):
                            ps = opp.tile([128, TC], F32, tag="op")
                            for cc in range(N_GCB):
                                nc.tensor.matmul(
                                    ps[:],
                                    aot[:, cc, tb * 128:(tb + 1) * 128],
                                    wos[:, cc, oc * TC:(oc + 1) * TC],
                                    start=(cc == 0),
                                    stop=(cc == N_GCB - 1),
                                )
                            nc.vector.tensor_copy(
                                ot[:, oc * TC:(oc + 1) * TC], ps[:]
                            )
                        nc.sync.dma_start(out[tb * 128:(tb + 1) * 128, :], ot[:])
            lctx.__exit__(None, None, None)

    nc.compile()
    return nc


_CACHE = {}


def _make_masks():
    m = np.zeros((4, KB, TC), np.float32)
    for i in range(4):
        for dk in range(KB):
            m[i, dk, KB * i + dk:] = 1.0
    return m.astype(ml_dtypes.bfloat16)


def _make_sel():
    s = np.zeros((32, 32 * Dh), np.float32)
    for i in range(32):
        s[i, i * Dh:(i + 1) * Dh] = 1.0
    return s


def make_in_maps(x, W_qkv, W_out):
    masks = _make_masks()
    in_maps = []
    for core in range(N_CORES):
        b, g = divmod(core, G)
        cs = slice(g * GC, (g + 1) * GC)
        in_maps.append({
            "xT": np.ascontiguousarray(x[b].T).astype(_BF),
            "wq": np.ascontiguousarray(W_qkv[:, cs]).astype(_BF),
            "wk": np.ascontiguousarray(
                W_qkv[:, C + g * GC:C + (g + 1) * GC]).astype(_BF),
            "wv": np.ascontiguousarray(
                W_qkv[:, 2 * C + g * GC:2 * C + (g + 1) * GC]).astype(_BF),
            "wo": np.ascontiguousarray(W_out[cs, :]).astype(_BF),
            "masks": masks,
            "ones": np.ones((1, Dh), np.float32),
            "sel": _make_sel(),
        })
    return in_maps


def kernel(x, W_qkv, W_out):
    x = np.ascontiguousarray(np.asarray(x, dtype=np.float32))
    W_qkv = np.asarray(W_qkv, dtype=np.float32)
    W_out = np.asarray(W_out, dtype=np.float32)

    if "nc" not in _CACHE:
        _CACHE["nc"] = build_program()
    nc = _CACHE["nc"]

    in_maps = make_in_maps(x, W_qkv, W_out)
    res = bass_utils.run_bass_kernel_spmd(nc, in_maps, core_ids=list(range(N_CORES)))

    out = np.empty((B, T, C), np.float32)
    for b in range(B):
        out[b] = res.results[G * b]["out"]
        for g in range(1, G):
            out[b] += res.results[G * b + g]["out"]
    return out



# revision 2
# speedup vs baseline: 1.1557x; 1.1557x over previous
"""Multi-head causal attention on 8 Trainium2 NeuronCores.

Sharding: data-parallel over batch (4) x tensor-parallel over heads (2 groups
of 8 heads). Each core computes a partial output [T, C] for one batch element
using its 8 heads; the host sums the two partials per batch element (the
"all-reduce after out_proj" done during unshard).

v2 design notes (vs the phase-separated baseline):
  - All inputs host-pre-arranged so every DMA is contiguous per partition,
    ordered so the first projection matmul can start at ~4.5us.
  - One interleaved instruction stream: projection work units for token
    chunk t+1 and the normalize+out-proj tail for chunk j-1 are emitted
    between attention head-pairs of chunk j, so the PE never idles long
    enough for the HAM clock gate to re-throttle and the scalar engine's
    exp stream always has matmul work to hide behind.
  - Causal staircase computed at partial width: for key block kb of query
    chunk j, only queries >= kb*128 are computed (saves ~25% of score/AV
    matmul columns and exp columns); only the leading 128 columns of a
    diagonal block need the triangular mask multiply.
  - Denominators (ones-row of the augmented V matmul) staged per chunk,
    reciprocal'd per chunk, broadcast via a tiny f32r selection matmul, and
    applied in-place to attn_outT right before that chunk's out-projection.
  - Output written bf16 (halves writeback); host upcasts and sums partials.

Per-core layouts (partition dim first):
  qt/kt/aot [128, 4, 2048]: partition = (head%2)*64 + d, dim1 = head//2 (pair)
  vaug [128, 16, 8, 65] bf16: partition = key-in-block, ones-augmented col 64
  scores^T per (pair, kb): psum [128, 2, 512] = key x (half, query)
"""

import numpy as np
import ml_dtypes

_BF = ml_dtypes.bfloat16

import concourse.bass as bass
import concourse.bacc as bacc
import concourse.mybir as mybir
import concourse.tile as tile
from concourse import bass_utils

F32 = mybir.dt.float32
F32R = mybir.dt.float32r
BF16 = mybir.dt.bfloat16

B, T, C = 4, 2048, 1024
H, Dh = 16, 64
G = 2                 # head groups (tensor parallel)
HPG = H // G          # 8 heads per group
GC = HPG * Dh         # group channels = 512
N_CORES = 8
TC = 512              # token chunk
KB = 128              # key block
N_TC = T // TC        # 4
N_KB = T // KB        # 16
N_CC = C // 128       # contraction chunks over C = 8
N_GCB = GC // 128     # head pairs = 4


def build_program():
    nc = bacc.Bacc("TRN2", target_bir_lowering=False, debug=False)

    xT = nc.dram_tensor("xT", [N_TC, 128, N_CC, TC], BF16, kind="ExternalInput").ap()
    wq = nc.dram_tensor("wq", [128, N_GCB, N_CC, 128], BF16, kind="ExternalInput").ap()
    wk = nc.dram_tensor("wk", [128, N_GCB, N_CC, 128], BF16, kind="ExternalInput").ap()
    wv = nc.dram_tensor("wv", [128, N_CC, GC], BF16, kind="ExternalInput").ap()
    wo = nc.dram_tensor("wo", [128, N_GCB, C], BF16, kind="ExternalInput").ap()
    masks = nc.dram_tensor("masks", [KB, 2, KB], BF16, kind="ExternalInput").ap()
    sel_in = nc.dram_tensor("sel", [8, N_GCB, 128], F32R, kind="ExternalInput").ap()
    out = nc.dram_tensor("out", [T, C], BF16, kind="ExternalOutput").ap()

    EXP = mybir.ActivationFunctionType.Exp

    with tile.TileContext(nc) as tc:
        with (
            tc.tile_pool(name="persist", bufs=1) as pp,
            tc.tile_pool(name="xp", bufs=2) as xp,
            tc.tile_pool(name="pr_pool", bufs=4) as prp,
            tc.tile_pool(name="ot_pool", bufs=3) as otp,
            tc.tile_pool(name="dn_pool", bufs=2) as dnp,
            tc.tile_pool(name="sc_psum", bufs=2, space="PSUM") as scp,
            tc.tile_pool(name="av_psum", bufs=1, space="PSUM") as avp,
            tc.tile_pool(name="ps_psum", bufs=2, space="PSUM") as psp,
        ):
            qt = pp.tile([128, N_GCB, T], BF16)
            kt = pp.tile([128, N_GCB, T], BF16)
            vaug = pp.tile([128, N_KB, HPG, Dh + 1], BF16)
            aot = pp.tile([128, N_GCB, T], BF16)
            msk = pp.tile([KB, 2, KB], BF16)
            sel = pp.tile([8, N_GCB, 128], F32R)
            wqs = pp.tile([128, N_GCB, N_CC, 128], BF16)
            wks = pp.tile([128, N_GCB, N_CC, 128], BF16)
            wvs = pp.tile([128, N_CC, GC], BF16)
            wos = pp.tile([128, N_GCB, C], BF16)

            # ---- input DMAs, ordered for earliest compute start ----------
            nc.sync.dma_start(msk[:], masks)
            nc.sync.dma_start(sel[:], sel_in)
            xts = [None] * N_TC

            def dma_x(t):
                xts[t] = xp.tile([128, N_CC, TC], BF16, tag="xt", name=f"xt{t}")
                nc.sync.dma_start(xts[t][:], xT[t])

            dma_x(0)
            for oc in range(N_GCB):
                nc.sync.dma_start(wqs[:, oc], wq[:, oc])
            for oc in range(N_GCB):
                nc.sync.dma_start(wks[:, oc], wk[:, oc])
            nc.sync.dma_start(wvs[:], wv)
            nc.vector.memset(vaug[:, :, :, Dh:], 1.0)

            # ---- qkv projection work units for token chunk t -------------
            def unit_qk(t, oc, w_s, dst):
                ps = psp.tile([128, TC], F32, tag="ps", name="pjq")
                for kc in range(N_CC):
                    nc.tensor.matmul(
                        ps[:], w_s[:, oc, kc], xts[t][:, kc],
                        start=(kc == 0), stop=(kc == N_CC - 1),
                    )
                nc.vector.tensor_copy(dst[:, oc, t * TC:(t + 1) * TC], ps[:])

            def unit_v(t, tb):
                ps = psp.tile([128, GC], F32, tag="ps", name="pjv")
                for kc in range(N_CC):
                    nc.tensor.matmul(
                        ps[:], xts[t][:, kc, tb * 128:(tb + 1) * 128],
                        wvs[:, kc],
                        start=(kc == 0), stop=(kc == N_CC - 1),
                    )
                nc.vector.tensor_copy(
                    vaug[:, t * 4 + tb, :, :Dh],
                    ps.rearrange("p (h d) -> p h d", h=HPG),
                )

            def phase2_units(t):
                us = []
                for oc in range(N_GCB):
                    us.append(lambda oc=oc: unit_qk(t, oc, wqs, qt))
                    us.append(lambda oc=oc: unit_qk(t, oc, wks, kt))
                for tb in range(4):
                    us.append(lambda tb=tb: unit_v(t, tb))
                return us

            dens_t = [None] * N_TC

            # ---- attention for (query chunk j, head pair p) --------------
            def attn_pair(j, p):
                av = avp.tile([Dh + 1, 2, TC], F32, tag="av", name="av")
                nkb = 4 * j + 4
                for kb in range(nkb):
                    off = KB * (kb - 4 * j) if kb >= 4 * j else 0
                    sc = scp.tile([128, 2, TC], F32, tag="sc", name="sc")
                    for half in range(2):
                        p0 = half * Dh
                        nc.tensor.matmul(
                            sc[:, half, off:],
                            kt[p0:p0 + Dh, p, kb * KB:(kb + 1) * KB],
                            qt[p0:p0 + Dh, p, j * TC + off:(j + 1) * TC],
                            start=True, stop=True,
                        )
                    pr = prp.tile([128, 2, TC], BF16, tag="pr", name="pr")
                    nc.scalar.activation(pr[:, :, off:], sc[:, :, off:], EXP)
                    if kb >= 4 * j:
                        nc.vector.tensor_mul(
                            pr[:, :, off:off + KB], pr[:, :, off:off + KB],
                            msk[:],
                        )
                    for half in range(2):
                        nc.tensor.matmul(
                            av[:, half, off:],
                            vaug[:, kb, 2 * p + half],
                            pr[:, half, off:],
                            start=(kb == 0), stop=(kb == nkb - 1),
                            skip_group_check=True,
                        )
                for half in range(2):
                    p0 = half * Dh
                    nc.vector.tensor_copy(
                        aot[p0:p0 + Dh, p, j * TC:(j + 1) * TC], av[:Dh, half]
                    )
                    s = 2 * p + half
                    nc.vector.tensor_copy(
                        dens_t[j][32 * (s // 3):32 * (s // 3) + 1, s % 3],
                        av[Dh:Dh + 1, half],
                    )

            # ---- normalize + out-projection for query chunk j ------------
            def tail(j):
                dcomp = dnp.tile([8, TC], F32, tag="dc", name="dcomp")
                for a in range(3):
                    cnt = 3 if a < 2 else 2
                    nc.sync.dma_start(
                        dcomp[3 * a:3 * a + cnt],
                        dens_t[j][32 * a:32 * a + 1, 0:cnt],
                    )
                rec = dnp.tile([8, TC], F32R, tag="rec", name="rec")
                with nc.allow_low_precision(
                    reason="fp32r reciprocal feeds bcast matmul"
                ):
                    nc.vector.reciprocal(rec[:], dcomp[:])
                for p in range(N_GCB):
                    bc = psp.tile([128, TC], F32, tag="ps", name="bc")
                    nc.tensor.matmul(bc[:], sel[:, p], rec[:], start=True, stop=True)
                    nc.vector.tensor_mul(
                        aot[:, p, j * TC:(j + 1) * TC],
                        aot[:, p, j * TC:(j + 1) * TC],
                        bc[:],
                    )
                for tb in range(4 * j, 4 * j + 4):
                    ot = otp.tile([128, C], BF16, tag="ot", name="ot")
                    for oc in range(2):
                        ps = psp.tile([128, TC], F32, tag="ps", name="op")
                        for cc in range(N_GCB):
                            nc.tensor.matmul(
                                ps[:],
                                aot[:, cc, tb * 128:(tb + 1) * 128],
                                wos[:, cc, oc * TC:(oc + 1) * TC],
                                start=(cc == 0), stop=(cc == N_GCB - 1),
                            )
                        nc.vector.tensor_copy(ot[:, oc * TC:(oc + 1) * TC], ps[:])
                    nc.sync.dma_start(out[tb * 128:(tb + 1) * 128], ot[:])

            # ---- interleaved schedule ------------------------------------
            for u in phase2_units(0):
                u()

            for j in range(N_TC):
                dens_t[j] = dnp.tile([65, 3, TC], F32, tag="dens", name=f"dens{j}")
                if j + 1 < N_TC:
                    dma_x(j + 1)
                if j == 0:
                    nc.sync.dma_start(wos[:], wo)
                queue = phase2_units(j + 1) if j + 1 < N_TC else []
                for p in range(N_GCB):
                    attn_pair(j, p)
                    if p == 0 and j >= 1:
                        tail(j - 1)
                    for u in queue[3 * p:3 * p + 3]:
                        u()
            tail(N_TC - 1)

    nc.compile()
    return nc


_CACHE = {}


def _make_masks():
    m = np.zeros((KB, 2, KB), np.float32)
    for dk in range(KB):
        m[dk, :, dk:] = 1.0
    return m.astype(_BF)


def _make_sel():
    s = np.zeros((8, N_GCB, 128), np.float32)
    for p in range(N_GCB):
        for m in range(128):
            s[2 * p + m // Dh, p, m] = 1.0
    return s


def make_in_maps(x, W_qkv, W_out):
    masks = _make_masks()
    sel = _make_sel()
    in_maps = []
    for core in range(N_CORES):
        b, g = divmod(core, G)
        cs = slice(g * GC, (g + 1) * GC)
        xt_arr = np.ascontiguousarray(
            x[b].T.reshape(N_CC, 128, N_TC, TC).transpose(2, 1, 0, 3)
        ).astype(_BF)
        wq_l = np.ascontiguousarray(
            (W_qkv[:, cs] * 0.125)
            .reshape(N_CC, 128, N_GCB, 2, Dh)
            .transpose(1, 2, 0, 3, 4)
            .reshape(128, N_GCB, N_CC, 128)
        ).astype(_BF)
        wk_l = np.ascontiguousarray(
            W_qkv[:, C + g * GC:C + (g + 1) * GC]
            .reshape(N_CC, 128, N_GCB, 2, Dh)
            .transpose(1, 2, 0, 3, 4)
            .reshape(128, N_GCB, N_CC, 128)
        ).astype(_BF)
        wv_l = np.ascontiguousarray(
            W_qkv[:, 2 * C + g * GC:2 * C + (g + 1) * GC]
            .reshape(N_CC, 128, GC)
            .transpose(1, 0, 2)
        ).astype(_BF)
        wo_l = np.ascontiguousarray(
            W_out[cs, :]
            .reshape(N_GCB, 2, Dh, C)
            .transpose(1, 2, 0, 3)
            .reshape(128, N_GCB, C)
        ).astype(_BF)
        in_maps.append({
            "xT": xt_arr,
            "wq": wq_l,
            "wk": wk_l,
            "wv": wv_l,
            "wo": wo_l,
            "masks": masks,
            "sel": sel,
        })
    return in_maps


def kernel(x, W_qkv, W_out):
    x = np.ascontiguousarray(np.asarray(x, dtype=np.float32))
    W_qkv = np.asarray(W_qkv, dtype=np.float32)
    W_out = np.asarray(W_out, dtype=np.float32)

    if "nc" not in _CACHE:
        _CACHE["nc"] = build_program()
    nc = _CACHE["nc"]

    in_maps = make_in_maps(x, W_qkv, W_out)
    res = bass_utils.run_bass_kernel_spmd(nc, in_maps, core_ids=list(range(N_CORES)))

    out = np.empty((B, T, C), np.float32)
    for b in range(B):
        acc = res.results[G * b]["out"].astype(np.float32)
        for g in range(1, G):
            acc = acc + res.results[G * b + g]["out"].astype(np.float32)
        out[b] = acc
    return out


# revision 7
# speedup vs baseline: 1.2101x; 1.0471x over previous
"""Multi-head causal attention on 8 Trainium2 NeuronCores.

Sharding: data-parallel over batch (4) x tensor-parallel over heads (2 groups
of 8 heads). Each core computes a partial output [T, C] for one batch element
using its 8 heads; the host sums the two partials per batch element (the
"all-reduce after out_proj" done during unshard).

v3 design notes:
  - Inputs host-pre-arranged so every DMA is contiguous per partition; x
    chunk 0 goes on the scalar DMA queue and weights on the sync queue so
    descriptor generation overlaps and the first matmul starts ~9us.
  - One interleaved instruction stream: projection work units for token
    chunk t+1, the out-projection for chunk j-1, and per-pair softmax
    normalization are emitted between attention head-pairs of chunk j, so
    the PE never idles long enough for the HAM clock gate to re-throttle.
  - Causal staircase computed at partial width: for key block kb of query
    chunk j only queries >= kb*128 are computed (saves ~25% of score/AV
    matmul columns and exp columns); only the leading 128 columns of a
    diagonal block need the triangular mask multiply.
  - Denominators (ones-row of the augmented V matmul) land at partition
    32*p of a staging tile, are reciprocal'd per head-pair with the 1-op
    ~51-ULP approx reciprocal, broadcast via a tiny K=2 f32r matmul, and
    applied in-place to attn_outT immediately — no serial tail chain.
  - Output written bf16 (halves writeback); host upcasts and sums partials.

Per-core layouts (partition dim first):
  qt/kt/aot [128, 4, 2048]: partition = (head%2)*64 + d, dim1 = head//2 (pair)
  vaug [128, 16, 8, 65] bf16: partition = key-in-block, ones-augmented col 64
  scores^T per (pair, kb): psum [128, 2, 512] = key x (half, query)
"""

import numpy as np
import ml_dtypes

_BF = ml_dtypes.bfloat16

import concourse.bass as bass
import concourse.bacc as bacc
import concourse.mybir as mybir
import concourse.tile as tile
from concourse import bass_utils

F32 = mybir.dt.float32
F32R = mybir.dt.float32r
BF16 = mybir.dt.bfloat16

B, T, C = 4, 2048, 1024
H, Dh = 16, 64
G = 2                 # head groups (tensor parallel)
HPG = H // G          # 8 heads per group
GC = HPG * Dh         # group channels = 512
N_CORES = 8
TC = 512              # token chunk
KB = 128              # key block
N_TC = T // TC        # 4
N_KB = T // KB        # 16
N_CC = C // 128       # contraction chunks over C = 8
N_GCB = GC // 128     # head pairs = 4


def build_program():
    nc = bacc.Bacc("TRN2", target_bir_lowering=False, debug=False)

    xT = nc.dram_tensor("xT", [N_TC, 128, N_CC, TC], BF16, kind="ExternalInput").ap()
    wq = nc.dram_tensor("wq", [128, N_GCB, N_CC, 128], BF16, kind="ExternalInput").ap()
    wk = nc.dram_tensor("wk", [128, N_GCB, N_CC, 128], BF16, kind="ExternalInput").ap()
    wv = nc.dram_tensor("wv", [128, N_CC, GC], BF16, kind="ExternalInput").ap()
    wo = nc.dram_tensor("wo", [128, N_GCB, C], BF16, kind="ExternalInput").ap()
    masks = nc.dram_tensor("masks", [KB, 2, KB], BF16, kind="ExternalInput").ap()
    sel_in = nc.dram_tensor("sel", [2, 128], BF16, kind="ExternalInput").ap()
    out = nc.dram_tensor("out", [T, C], BF16, kind="ExternalOutput").ap()

    EXP = mybir.ActivationFunctionType.Exp

    with tile.TileContext(nc) as tc:
        with (
            tc.tile_pool(name="persist", bufs=1) as pp,
            tc.tile_pool(name="xp", bufs=2) as xp,
            tc.tile_pool(name="pr_pool", bufs=4) as prp,
            tc.tile_pool(name="ot_pool", bufs=3) as otp,
            tc.tile_pool(name="dn_pool", bufs=2) as dnp,
            tc.tile_pool(name="sc_psum", bufs=2, space="PSUM") as scp,
            tc.tile_pool(name="av_psum", bufs=1, space="PSUM") as avp,
            tc.tile_pool(name="ps_psum", bufs=2, space="PSUM") as psp,
        ):
            qt = pp.tile([128, N_GCB, T], BF16)
            kt = pp.tile([128, N_GCB, T], BF16)
            vaug = pp.tile([128, N_KB, HPG, Dh + 1], BF16)
            aot = pp.tile([128, N_GCB, T], BF16)
            msk = pp.tile([KB, 2, KB], BF16)
            sel = pp.tile([2, 128], BF16)
            wqs = pp.tile([128, N_GCB, N_CC, 128], BF16)
            wks = pp.tile([128, N_GCB, N_CC, 128], BF16)
            wvs = pp.tile([128, N_CC, GC], BF16)
            wos = pp.tile([128, N_GCB, C], BF16)

            # ---- input DMAs: x chunk 0 on the scalar queue, weights on ---
            # ---- sync, so desc-gen and transfers overlap -----------------
            xts = [None] * N_TC

            def dma_x(t, eng=None):
                xts[t] = xp.tile([128, N_CC, TC], BF16, tag="xt", name=f"xt{t}")
                (eng or nc.sync).dma_start(xts[t][:], xT[t])

            dma_x(0, nc.scalar)
            nc.scalar.dma_start(msk[:], masks)
            nc.scalar.dma_start(sel[:], sel_in)
            for oc in range(N_GCB):
                nc.sync.dma_start(wqs[:, oc], wq[:, oc])
            for oc in range(N_GCB):
                nc.sync.dma_start(wks[:, oc], wk[:, oc])
            nc.sync.dma_start(wvs[:], wv)
            nc.vector.memset(vaug[:, :, :, Dh:], 1.0)

            # ---- qkv projection work units for token chunk t -------------
            def unit_qk(t, oc, w_s, dst):
                ps = psp.tile([128, TC], F32, tag="ps", name="pjq")
                for kc in range(N_CC):
                    nc.tensor.matmul(
                        ps[:], w_s[:, oc, kc], xts[t][:, kc],
                        start=(kc == 0), stop=(kc == N_CC - 1),
                    )
                nc.vector.tensor_copy(dst[:, oc, t * TC:(t + 1) * TC], ps[:])

            def unit_v(t, tb):
                ps = psp.tile([128, GC], F32, tag="ps", name="pjv")
                for kc in range(N_CC):
                    nc.tensor.matmul(
                        ps[:], xts[t][:, kc, tb * 128:(tb + 1) * 128],
                        wvs[:, kc],
                        start=(kc == 0), stop=(kc == N_CC - 1),
                    )
                nc.vector.tensor_copy(
                    vaug[:, t * 4 + tb, :, :Dh],
                    ps.rearrange("p (h d) -> p h d", h=HPG),
                )

            def phase2_units(t):
                us = []
                for oc in range(N_GCB):
                    us.append(lambda oc=oc: unit_qk(t, oc, wqs, qt))
                    us.append(lambda oc=oc: unit_qk(t, oc, wks, kt))
                for tb in range(4):
                    us.append(lambda tb=tb: unit_v(t, tb))
                return us

            rec_t = [None] * (N_TC * N_GCB)

            # ---- attention + fused normalize for (chunk j, head pair p) --
            def attn_pair(j, p):
                av = avp.tile([Dh + 1, 2, TC], F32, tag="av", name="av")
                nkb = 4 * j + 4
                for kb in range(nkb):
                    off = KB * (kb - 4 * j) if kb >= 4 * j else 0
                    sc = scp.tile([128, 2, TC], F32, tag="sc", name="sc")
                    for half in range(2):
                        p0 = half * Dh
                        nc.tensor.matmul(
                            sc[:, half, off:],
                            kt[p0:p0 + Dh, p, kb * KB:(kb + 1) * KB],
                            qt[p0:p0 + Dh, p, j * TC + off:(j + 1) * TC],
                            start=True, stop=True,
                        )
                    pr = prp.tile([128, 2, TC], BF16, tag="pr", name="pr")
                    nc.scalar.activation(pr[:, :, off:], sc[:, :, off:], EXP)
                    if kb >= 4 * j:
                        nc.vector.tensor_mul(
                            pr[:, :, off:off + KB], pr[:, :, off:off + KB],
                            msk[:],
                        )
                    for half in range(2):
                        nc.tensor.matmul(
                            av[:, half, off:],
                            vaug[:, kb, 2 * p + half],
                            pr[:, half, off:],
                            start=(kb == 0), stop=(kb == nkb - 1),
                            skip_group_check=True,
                        )
                # drain attn output + denominators, then normalize in place
                for half in range(2):
                    p0 = half * Dh
                    nc.vector.tensor_copy(
                        aot[p0:p0 + Dh, p, j * TC:(j + 1) * TC], av[:Dh, half]
                    )
                dn = dnp.tile([1, 2, TC], F32, tag="dn", name="dn", bufs=8)
                nc.vector.tensor_copy(dn[:], av[Dh:Dh + 1])
                dc = dnp.tile([2, TC], F32, tag="dc", name="dc", bufs=8)
                nc.sync.dma_start(dc[:], dn[0:1])
                rc = dnp.tile([2, TC], F32, tag="rc", name="rc", bufs=8)
                nc.vector.reciprocal_approx_fast(rc[:], dc[:])
                rcb = dnp.tile([2, TC], BF16, tag="rcb", name="rcb", bufs=8)
                rec_t[4 * j + p] = rcb
                nc.vector.tensor_copy(rcb[:], rc[:])

            # ---- softmax divide for (chunk j, head pair p), scheduled ----
            # ---- one pair later so the PE never waits on the recip chain -
            def norm_pair(j, p):
                rc = rec_t[4 * j + p]  # bf16 reciprocal rows
                bc = psp.tile([128, TC], F32, tag="ps", name="bc")
                nc.tensor.matmul(
                    bc[:], sel[:], rc[:],
                    start=True, stop=True,
                )
                nc.vector.tensor_mul(
                    aot[:, p, j * TC:(j + 1) * TC],
                    aot[:, p, j * TC:(j + 1) * TC],
                    bc[:],
                )

            # ---- out-projection for query chunk j ------------------------
            def tail_op(j):
                for tb in range(4 * j, 4 * j + 4):
                    ot = otp.tile([128, C], BF16, tag="ot", name="ot")
                    for oc in range(2):
                        ps = psp.tile([128, TC], F32, tag="ps", name="op")
                        for cc in range(N_GCB):
                            nc.tensor.matmul(
                                ps[:],
                                aot[:, cc, tb * 128:(tb + 1) * 128],
                                wos[:, cc, oc * TC:(oc + 1) * TC],
                                start=(cc == 0), stop=(cc == N_GCB - 1),
                            )
                        nc.vector.tensor_copy(ot[:, oc * TC:(oc + 1) * TC], ps[:])
                    nc.sync.dma_start(out[tb * 128:(tb + 1) * 128], ot[:])

            # ---- interleaved schedule ------------------------------------
            for u in phase2_units(0):
                u()

            for j in range(N_TC):
                if j + 1 < N_TC:
                    dma_x(j + 1)
                if j == 0:
                    nc.sync.dma_start(wos[:], wo)
                queue = phase2_units(j + 1) if j + 1 < N_TC else []
                for p in range(N_GCB):
                    attn_pair(j, p)
                    if p >= 1:
                        norm_pair(j, p - 1)
                    if p == 0 and j >= 1:
                        tail_op(j - 1)
                    for u in queue[3 * p:3 * p + 3]:
                        u()
                norm_pair(j, 3)
            tail_op(N_TC - 1)

    nc.compile()
    return nc


_CACHE = {}


def _make_masks():
    m = np.zeros((KB, 2, KB), np.float32)
    for dk in range(KB):
        m[dk, :, dk:] = 1.0
    return m.astype(_BF)


def _make_sel():
    s = np.zeros((2, 128), np.float32)
    for m in range(128):
        s[m // Dh, m] = 1.0
    return s.astype(_BF)


def make_in_maps(x, W_qkv, W_out):
    masks = _make_masks()
    sel = _make_sel()
    in_maps = []
    for core in range(N_CORES):
        b, g = divmod(core, G)
        cs = slice(g * GC, (g + 1) * GC)
        xt_arr = np.ascontiguousarray(
            x[b].T.reshape(N_CC, 128, N_TC, TC).transpose(2, 1, 0, 3)
        ).astype(_BF)
        wq_l = np.ascontiguousarray(
            (W_qkv[:, cs] * 0.125)
            .reshape(N_CC, 128, N_GCB, 2, Dh)
            .transpose(1, 2, 0, 3, 4)
            .reshape(128, N_GCB, N_CC, 128)
        ).astype(_BF)
        wk_l = np.ascontiguousarray(
            W_qkv[:, C + g * GC:C + (g + 1) * GC]
            .reshape(N_CC, 128, N_GCB, 2, Dh)
            .transpose(1, 2, 0, 3, 4)
            .reshape(128, N_GCB, N_CC, 128)
        ).astype(_BF)
        wv_l = np.ascontiguousarray(
            W_qkv[:, 2 * C + g * GC:2 * C + (g + 1) * GC]
            .reshape(N_CC, 128, GC)
            .transpose(1, 0, 2)
        ).astype(_BF)
        wo_l = np.ascontiguousarray(
            W_out[cs, :]
            .reshape(N_GCB, 2, Dh, C)
            .transpose(1, 2, 0, 3)
            .reshape(128, N_GCB, C)
        ).astype(_BF)
        in_maps.append({
            "xT": xt_arr,
            "wq": wq_l,
            "wk": wk_l,
            "wv": wv_l,
            "wo": wo_l,
            "masks": masks,
            "sel": sel,
        })
    return in_maps


def kernel(x, W_qkv, W_out):
    x = np.ascontiguousarray(np.asarray(x, dtype=np.float32))
    W_qkv = np.asarray(W_qkv, dtype=np.float32)
    W_out = np.asarray(W_out, dtype=np.float32)

    if "nc" not in _CACHE:
        _CACHE["nc"] = build_program()
    nc = _CACHE["nc"]

    in_maps = make_in_maps(x, W_qkv, W_out)
    res = bass_utils.run_bass_kernel_spmd(nc, in_maps, core_ids=list(range(N_CORES)))

    out = np.empty((B, T, C), np.float32)
    for b in range(B):
        acc = res.results[G * b]["out"].astype(np.float32)
        for g in range(1, G):
            acc = acc + res.results[G * b + g]["out"].astype(np.float32)
        out[b] = acc
    return out


# revision 8
# speedup vs baseline: 1.2128x; 1.0022x over previous
"""Multi-head causal attention on 8 Trainium2 NeuronCores.

Sharding: data-parallel over batch (4) x tensor-parallel over heads (2 groups
of 8 heads). Each core computes a partial output [T, C] for one batch element
using its 8 heads; the host sums the two partials per batch element (the
"all-reduce after out_proj" done during unshard).

v3 design notes:
  - Inputs host-pre-arranged so every DMA is contiguous per partition; x
    chunk 0 goes on the scalar DMA queue and weights on the sync queue so
    descriptor generation overlaps and the first matmul starts ~9us.
  - One interleaved instruction stream: projection work units for token
    chunk t+1, the out-projection for chunk j-1, and per-pair softmax
    normalization are emitted between attention head-pairs of chunk j, so
    the PE never idles long enough for the HAM clock gate to re-throttle.
  - Causal staircase computed at partial width: for key block kb of query
    chunk j only queries >= kb*128 are computed (saves ~25% of score/AV
    matmul columns and exp columns); only the leading 128 columns of a
    diagonal block need the triangular mask multiply.
  - Denominators (ones-row of the augmented V matmul) land at partition
    32*p of a staging tile, are reciprocal'd per head-pair with the 1-op
    ~51-ULP approx reciprocal, broadcast via a tiny K=2 f32r matmul, and
    applied in-place to attn_outT immediately — no serial tail chain.
  - Output written bf16 (halves writeback); host upcasts and sums partials.

Per-core layouts (partition dim first):
  qt/kt/aot [128, 4, 2048]: partition = (head%2)*64 + d, dim1 = head//2 (pair)
  vaug [128, 16, 8, 65] bf16: partition = key-in-block, ones-augmented col 64
  scores^T per (pair, kb): psum [128, 2, 512] = key x (half, query)
"""

import numpy as np
import ml_dtypes

_BF = ml_dtypes.bfloat16

import concourse.bass as bass
import concourse.bacc as bacc
import concourse.mybir as mybir
import concourse.tile as tile
from concourse import bass_utils

F32 = mybir.dt.float32
F32R = mybir.dt.float32r
BF16 = mybir.dt.bfloat16

B, T, C = 4, 2048, 1024
H, Dh = 16, 64
G = 2                 # head groups (tensor parallel)
HPG = H // G          # 8 heads per group
GC = HPG * Dh         # group channels = 512
N_CORES = 8
TC = 512              # token chunk
KB = 128              # key block
N_TC = T // TC        # 4
N_KB = T // KB        # 16
N_CC = C // 128       # contraction chunks over C = 8
N_GCB = GC // 128     # head pairs = 4


def build_program():
    nc = bacc.Bacc("TRN2", target_bir_lowering=False, debug=False)

    xT = nc.dram_tensor("xT", [N_TC, 128, N_CC, TC], BF16, kind="ExternalInput").ap()
    wq = nc.dram_tensor("wq", [128, N_GCB, N_CC, 128], BF16, kind="ExternalInput").ap()
    wk = nc.dram_tensor("wk", [128, N_GCB, N_CC, 128], BF16, kind="ExternalInput").ap()
    wv = nc.dram_tensor("wv", [128, N_CC, GC], BF16, kind="ExternalInput").ap()
    wo = nc.dram_tensor("wo", [128, N_GCB, C], BF16, kind="ExternalInput").ap()
    masks = nc.dram_tensor("masks", [KB, 2, KB], BF16, kind="ExternalInput").ap()
    sel_in = nc.dram_tensor("sel", [2, 128], BF16, kind="ExternalInput").ap()
    out = nc.dram_tensor("out", [T, C], BF16, kind="ExternalOutput").ap()

    EXP = mybir.ActivationFunctionType.Exp

    with tile.TileContext(nc) as tc:
        with (
            tc.tile_pool(name="persist", bufs=1) as pp,
            tc.tile_pool(name="xp", bufs=2) as xp,
            tc.tile_pool(name="pr_pool", bufs=4) as prp,
            tc.tile_pool(name="ot_pool", bufs=3) as otp,
            tc.tile_pool(name="dn_pool", bufs=2) as dnp,
            tc.tile_pool(name="sc_psum", bufs=2, space="PSUM") as scp,
            tc.tile_pool(name="av_psum", bufs=1, space="PSUM") as avp,
            tc.tile_pool(name="ps_psum", bufs=2, space="PSUM") as psp,
        ):
            qt = pp.tile([128, N_GCB, T], BF16)
            kt = pp.tile([128, N_GCB, T], BF16)
            vaug = pp.tile([128, N_KB, HPG, Dh + 1], BF16)
            aot = pp.tile([128, N_GCB, T], BF16)
            msk = pp.tile([KB, 2, KB], BF16)
            sel = pp.tile([2, 128], BF16)
            wqs = pp.tile([128, N_GCB, N_CC, 128], BF16)
            wks = pp.tile([128, N_GCB, N_CC, 128], BF16)
            wvs = pp.tile([128, N_CC, GC], BF16)
            wos = pp.tile([128, N_GCB, C], BF16)

            # ---- input DMAs: x chunk 0 on the scalar queue, weights on ---
            # ---- sync, so desc-gen and transfers overlap -----------------
            xts = [None] * N_TC

            def dma_x(t, eng=None):
                xts[t] = xp.tile([128, N_CC, TC], BF16, tag="xt", name=f"xt{t}")
                (eng or nc.sync).dma_start(xts[t][:], xT[t])

            dma_x(0, nc.scalar)
            for oc in range(N_GCB):
                nc.sync.dma_start(wqs[:, oc], wq[:, oc])
            for oc in range(N_GCB):
                nc.sync.dma_start(wks[:, oc], wk[:, oc])
            nc.sync.dma_start(wvs[:], wv)
            nc.sync.dma_start(msk[:], masks)
            nc.sync.dma_start(sel[:], sel_in)
            nc.vector.memset(vaug[:, :, :, Dh:], 1.0)

            # ---- qkv projection work units for token chunk t -------------
            def unit_qk(t, oc, w_s, dst):
                ps = psp.tile([128, TC], F32, tag="ps", name="pjq")
                for kc in range(N_CC):
                    nc.tensor.matmul(
                        ps[:], w_s[:, oc, kc], xts[t][:, kc],
                        start=(kc == 0), stop=(kc == N_CC - 1),
                    )
                nc.vector.tensor_copy(dst[:, oc, t * TC:(t + 1) * TC], ps[:])

            def unit_v(t, tb):
                ps = psp.tile([128, GC], F32, tag="ps", name="pjv")
                for kc in range(N_CC):
                    nc.tensor.matmul(
                        ps[:], xts[t][:, kc, tb * 128:(tb + 1) * 128],
                        wvs[:, kc],
                        start=(kc == 0), stop=(kc == N_CC - 1),
                    )
                nc.vector.tensor_copy(
                    vaug[:, t * 4 + tb, :, :Dh],
                    ps.rearrange("p (h d) -> p h d", h=HPG),
                )

            def phase2_units(t):
                us = []
                for oc in range(N_GCB):
                    us.append(lambda oc=oc: unit_qk(t, oc, wqs, qt))
                    us.append(lambda oc=oc: unit_qk(t, oc, wks, kt))
                for tb in range(4):
                    us.append(lambda tb=tb: unit_v(t, tb))
                return us

            rec_t = [None] * (N_TC * N_GCB)

            # ---- attention + fused normalize for (chunk j, head pair p) --
            def attn_pair(j, p):
                av = avp.tile([Dh + 1, 2, TC], F32, tag="av", name="av")
                nkb = 4 * j + 4
                for kb in range(nkb):
                    off = KB * (kb - 4 * j) if kb >= 4 * j else 0
                    sc = scp.tile([128, 2, TC], F32, tag="sc", name="sc")
                    for half in range(2):
                        p0 = half * Dh
                        nc.tensor.matmul(
                            sc[:, half, off:],
                            kt[p0:p0 + Dh, p, kb * KB:(kb + 1) * KB],
                            qt[p0:p0 + Dh, p, j * TC + off:(j + 1) * TC],
                            start=True, stop=True,
                        )
                    pr = prp.tile([128, 2, TC], BF16, tag="pr", name="pr")
                    nc.scalar.activation(pr[:, :, off:], sc[:, :, off:], EXP)
                    if kb >= 4 * j:
                        nc.vector.tensor_mul(
                            pr[:, :, off:off + KB], pr[:, :, off:off + KB],
                            msk[:],
                        )
                    for half in range(2):
                        nc.tensor.matmul(
                            av[:, half, off:],
                            vaug[:, kb, 2 * p + half],
                            pr[:, half, off:],
                            start=(kb == 0), stop=(kb == nkb - 1),
                            skip_group_check=True,
                        )
                # drain attn output + denominators, then normalize in place
                for half in range(2):
                    p0 = half * Dh
                    nc.vector.tensor_copy(
                        aot[p0:p0 + Dh, p, j * TC:(j + 1) * TC], av[:Dh, half]
                    )
                dn = dnp.tile([1, 2, TC], F32, tag="dn", name="dn", bufs=8)
                if j < 3:
                    nc.scalar.copy(dn[:], av[Dh:Dh + 1])
                else:
                    nc.vector.tensor_copy(dn[:], av[Dh:Dh + 1])
                dc = dnp.tile([2, TC], F32, tag="dc", name="dc", bufs=8)
                nc.sync.dma_start(dc[:], dn[0:1])
                rc = dnp.tile([2, TC], F32, tag="rc", name="rc", bufs=8)
                nc.vector.reciprocal_approx_fast(rc[:], dc[:])
                rcb = dnp.tile([2, TC], BF16, tag="rcb", name="rcb", bufs=8)
                rec_t[4 * j + p] = rcb
                nc.vector.tensor_copy(rcb[:], rc[:])

            # ---- softmax divide for (chunk j, head pair p), scheduled ----
            # ---- one pair later so the PE never waits on the recip chain -
            def norm_pair(j, p):
                rc = rec_t[4 * j + p]  # bf16 reciprocal rows
                bc = psp.tile([128, TC], F32, tag="ps", name="bc")
                nc.tensor.matmul(
                    bc[:], sel[:], rc[:],
                    start=True, stop=True,
                )
                nc.vector.tensor_mul(
                    aot[:, p, j * TC:(j + 1) * TC],
                    aot[:, p, j * TC:(j + 1) * TC],
                    bc[:],
                )

            # ---- out-projection for one token block ----------------------
            def tail_tb(tb):
                ot = otp.tile([128, C], BF16, tag="ot", name="ot")
                for oc in range(2):
                    ps = psp.tile([128, TC], F32, tag="ps", name="op")
                    for cc in range(N_GCB):
                        nc.tensor.matmul(
                            ps[:],
                            aot[:, cc, tb * 128:(tb + 1) * 128],
                            wos[:, cc, oc * TC:(oc + 1) * TC],
                            start=(cc == 0), stop=(cc == N_GCB - 1),
                        )
                    nc.vector.tensor_copy(ot[:, oc * TC:(oc + 1) * TC], ps[:])
                nc.sync.dma_start(out[tb * 128:(tb + 1) * 128], ot[:])

            # ---- interleaved schedule ------------------------------------
            for u in phase2_units(0):
                u()

            for j in range(N_TC):
                if j + 1 < N_TC:
                    dma_x(j + 1)
                if j == 0:
                    nc.sync.dma_start(wos[:], wo)
                queue = phase2_units(j + 1) if j + 1 < N_TC else []
                for p in range(N_GCB):
                    attn_pair(j, p)
                    if p >= 1:
                        norm_pair(j, p - 1)
                    if j >= 1:
                        tail_tb(4 * (j - 1) + p)
                    for u in queue[3 * p:3 * p + 3]:
                        u()
                norm_pair(j, 3)
            for tb in range(4 * (N_TC - 1), 4 * N_TC):
                tail_tb(tb)

    nc.compile()
    return nc


_CACHE = {}


def _make_masks():
    m = np.zeros((KB, 2, KB), np.float32)
    for dk in range(KB):
        m[dk, :, dk:] = 1.0
    return m.astype(_BF)


def _make_sel():
    s = np.zeros((2, 128), np.float32)
    for m in range(128):
        s[m // Dh, m] = 1.0
    return s.astype(_BF)


def make_in_maps(x, W_qkv, W_out):
    masks = _make_masks()
    sel = _make_sel()
    in_maps = []
    for core in range(N_CORES):
        b, g = divmod(core, G)
        cs = slice(g * GC, (g + 1) * GC)
        xt_arr = np.ascontiguousarray(
            x[b].T.reshape(N_CC, 128, N_TC, TC).transpose(2, 1, 0, 3)
        ).astype(_BF)
        wq_l = np.ascontiguousarray(
            (W_qkv[:, cs] * 0.125)
            .reshape(N_CC, 128, N_GCB, 2, Dh)
            .transpose(1, 2, 0, 3, 4)
            .reshape(128, N_GCB, N_CC, 128)
        ).astype(_BF)
        wk_l = np.ascontiguousarray(
            W_qkv[:, C + g * GC:C + (g + 1) * GC]
            .reshape(N_CC, 128, N_GCB, 2, Dh)
            .transpose(1, 2, 0, 3, 4)
            .reshape(128, N_GCB, N_CC, 128)
        ).astype(_BF)
        wv_l = np.ascontiguousarray(
            W_qkv[:, 2 * C + g * GC:2 * C + (g + 1) * GC]
            .reshape(N_CC, 128, GC)
            .transpose(1, 0, 2)
        ).astype(_BF)
        wo_l = np.ascontiguousarray(
            W_out[cs, :]
            .reshape(N_GCB, 2, Dh, C)
            .transpose(1, 2, 0, 3)
            .reshape(128, N_GCB, C)
        ).astype(_BF)
        in_maps.append({
            "xT": xt_arr,
            "wq": wq_l,
            "wk": wk_l,
            "wv": wv_l,
            "wo": wo_l,
            "masks": masks,
            "sel": sel,
        })
    return in_maps


def kernel(x, W_qkv, W_out):
    x = np.ascontiguousarray(np.asarray(x, dtype=np.float32))
    W_qkv = np.asarray(W_qkv, dtype=np.float32)
    W_out = np.asarray(W_out, dtype=np.float32)

    if "nc" not in _CACHE:
        _CACHE["nc"] = build_program()
    nc = _CACHE["nc"]

    in_maps = make_in_maps(x, W_qkv, W_out)
    res = bass_utils.run_bass_kernel_spmd(nc, in_maps, core_ids=list(range(N_CORES)))

    out = np.empty((B, T, C), np.float32)
    for b in range(B):
        acc = res.results[G * b]["out"].astype(np.float32)
        for g in range(1, G):
            acc = acc + res.results[G * b + g]["out"].astype(np.float32)
        out[b] = acc
    return out
